# revision 1
# baseline (speedup 1.0000x reference)
"""Sliding-window causal attention with RoPE, distributed over 8 NeuronCores.

Sharding: 8 cores = (batch b in {0,1}) x (head-group g in {0..3}); each core
computes its batch's attention for 4 heads (256 channels) plus that group's
partial of the output projection; the host sums the 4 partials per batch.

Per-core device pipeline (all layouts transposed so contraction dims sit on
SBUF partitions):
  qT/kT = wT.T @ xT (fp32r matmuls), RoPE applied in a de-interleaved channel
  layout so the pair swap is a stream_shuffle (rotate-16 within quadrants);
  v in token-major layout with a ones-column augment so the PV matmul also
  emits softmax denominators; scoresT = kT.T @ qT per 128-token k-tile;
  exp on ACT, banded mask via gpsimd affine_select (multiplicative, post-exp);
  oT = v_aug.T @ probsT (bf16); per-head 1/sum scaling fused into the
  psum->sbuf copy; output projection with pair-stacked oT (K=128) and fp16
  partial outputs.
"""

import numpy as np

B, T, D = 2, 2048, 1024
H, HD = 16, 64
G = 4            # head groups (cores per batch)
HPG = H // G     # heads per group = 4
C = HPG * HD     # channels per group = 256
SCALE = 0.125
W = 128          # window per side
NQ = 256         # query chunk
NCHUNK = T // NQ
KT = T // 128    # k tiles

_cache = {}


def _chan_perm():
    # within-head permutation: pair i=(2i,2i+1) -> block layout where rows
    # [0:16)=re(0..15), [16:32)=im(0..15), [32:48)=re(16..31), [48:64)=im(16..31)
    perm = np.zeros(HD, dtype=np.int64)
    for j in range(HD):
        if j < 16:
            perm[j] = 2 * j
        elif j < 32:
            perm[j] = 2 * (j - 16) + 1
        elif j < 48:
            perm[j] = 2 * (j - 16)
        else:
            perm[j] = 2 * (j - 32) + 1
    return perm


def _pair_of(j):
    return (j % 16) + 16 * (j // 32)


def _build_program(repeat=1):
    import concourse.mybir as mybir
    import concourse.tile as tile
    from concourse import bacc

    F32 = mybir.dt.float32
    F32R = mybir.dt.float32r
    BF16 = mybir.dt.bfloat16
    FP16 = mybir.dt.float16
    MULT = mybir.AluOpType.mult
    ADD = mybir.AluOpType.add
    GE = mybir.AluOpType.is_ge
    EXP = mybir.ActivationFunctionType.Exp

    nc = bacc.Bacc("TRN2", target_bir_lowering=False, debug=False, num_devices=8)

    xT = nc.dram_tensor("xT", (D, T), F32R, kind="ExternalInput")
    wqT = nc.dram_tensor("wqT", (D, C), F32R, kind="ExternalInput")
    wkT = nc.dram_tensor("wkT", (D, C), F32R, kind="ExternalInput")
    wvT = nc.dram_tensor("wvT", (D, C), F32R, kind="ExternalInput")
    woT = nc.dram_tensor("woT", (C, D), F32R, kind="ExternalInput")
    cosT = nc.dram_tensor("cosT", (128, T), F32, kind="ExternalInput")
    sinT = nc.dram_tensor("sinT", (128, T), F32, kind="ExternalInput")
    out = nc.dram_tensor("out", (T, D), FP16, kind="ExternalOutput")

    shuf16 = [(i + 16) % 32 for i in range(32)]

    with tile.TileContext(nc) as tc:
        with tc.tile_pool(name="persist", bufs=1) as pp:
            # ---- loads, ordered so the first projection chunk's operands
            # land first; one merged DMA per weight matrix
            wq_a = pp.tile([128, 8, C], F32R, tag="wq_a", name="wq_a")
            x_t = {}
            x_t[0, 0] = pp.tile([128, 512], F32R, tag="x0_0", name="x0_0")
            nc.sync.dma_start(x_t[0, 0][:], xT[0:128, 0:512])
            nc.sync.dma_start(wq_a[:, 0:4, :], wqT[0:512].rearrange("(g p) c -> p g c", p=128))
            for k in range(1, 8):
                x_t[k, 0] = pp.tile([128, 512], F32R, tag=f"x{k}_0", name=f"x{k}_0")
                nc.sync.dma_start(x_t[k, 0][:], xT[k * 128:(k + 1) * 128, 0:512])
            nc.sync.dma_start(wq_a[:, 4:8, :], wqT[512:1024].rearrange("(g p) c -> p g c", p=128))
            wq_t = [wq_a[:, k, :] for k in range(8)]
            wk_a = pp.tile([128, 8, C], F32R, tag="wk_a", name="wk_a")
            nc.sync.dma_start(wk_a[:], wkT.rearrange("(g p) c -> p g c", p=128))
            wk_t = [wk_a[:, k, :] for k in range(8)]
            cos_c, sin_c = [], []
            for c in range(4):
                cc = pp.tile([128, 512], F32, tag=f"cos{c}", name=f"cos{c}")
                sc_ = pp.tile([128, 512], F32, tag=f"sin{c}", name=f"sin{c}")
                if c == 0:
                    nc.sync.dma_start(cc[:], cosT[:, 0:512])
                    nc.sync.dma_start(sc_[:], sinT[:, 0:512])
                cos_c.append(cc)
                sin_c.append(sc_)
            wv_a = pp.tile([128, 8, C], F32R, tag="wv_a", name="wv_a")
            nc.sync.dma_start(wv_a[:], wvT.rearrange("(g p) c -> p g c", p=128))
            wv_t = [wv_a[:, k, :] for k in range(8)]
            for c in range(1, 4):
                nc.sync.dma_start(cos_c[c][:], cosT[:, c * 512:(c + 1) * 512])
                nc.sync.dma_start(sin_c[c][:], sinT[:, c * 512:(c + 1) * 512])
                for k in range(8):
                    x_t[k, c] = pp.tile([128, 512], F32R, tag=f"x{k}_{c}", name=f"x{k}_{c}")
                    nc.sync.dma_start(x_t[k, c][:], xT[k * 128:(k + 1) * 128,
                                                      c * 512:(c + 1) * 512])
            wo_a = pp.tile([128, 2, D], F32R, tag="wo_a", name="wo_a")
            nc.sync.dma_start(wo_a[:], woT.rearrange("(g p) c -> p g c", p=128))
            wo_t = [wo_a[:, k, :] for k in range(2)]

            # ---- persistent activation storage
            qr = {}
            kr = {}
            for m in range(2):
                for c in range(4):
                    qr[m, c] = pp.tile([128, 512], F32R, tag=f"qr{m}_{c}", name=f"qr{m}_{c}")
                    kr[m, c] = pp.tile([128, 512], F32R, tag=f"kr{m}_{c}", name=f"kr{m}_{c}")
            v_t = [pp.tile([128, 512], BF16, tag=f"v{t}", name=f"v{t}") for t in range(KT)]
            for t in range(KT):
                nc.gpsimd.memset(v_t[t][:], 1.0)

            for _rep in range(repeat):
                # ================= Phase B: projections =================
                with tc.tile_pool(name="projps", bufs=4, space="PSUM") as pjp, \
                     tc.tile_pool(name="projvps", bufs=3, space="PSUM") as pjv, \
                     tc.tile_pool(name="ropetmp", bufs=6) as rtp:
                    for c in range(4):
                        for (wt, dst) in ((wq_t, qr), (wk_t, kr)):
                            for m in range(2):
                                ps = pjp.tile([128, 512], F32, tag="proj", name="proj")
                                for k in range(8):
                                    nc.tensor.matmul(ps[:], wt[k][:, m * 128:(m + 1) * 128],
                                                     x_t[k, c][:],
                                                     start=(k == 0), stop=(k == 7))
                                cs = cos_c[c][:]
                                sn = sin_c[c][:]
                                z = rtp.tile([128, 512], F32, tag="ropez", name="ropez")
                                nc.vector.tensor_tensor(z[:], ps[:], sn, MULT)
                                zs = rtp.tile([128, 512], F32, tag="ropezs", name="ropezs")
                                nc.vector.stream_shuffle(zs[:], z[:], shuf16)
                                t1 = rtp.tile([128, 512], F32, tag="ropet1", name="ropet1")
                                nc.vector.tensor_tensor(t1[:], ps[:], cs, MULT)
                                nc.vector.tensor_tensor(dst[m, c][:], t1[:], zs[:], ADD)
                        # v projection for the 4 token-tiles of this chunk
                        for tt in range(c * 4, c * 4 + 4):
                            psv = pjv.tile([128, C], F32, tag="projv", name="projv")
                            for k in range(8):
                                nc.tensor.matmul(psv[:], x_t[k, c][:, (tt % 4) * 128:(tt % 4) * 128 + 128],
                                                 wv_t[k][:], start=(k == 0), stop=(k == 7))
                            src_ = psv[:].rearrange("p (h d) -> p h d", h=HPG)
                            dstv = v_t[tt][:].rearrange("p (h d) -> p h d", h=HPG)[:, :, 0:64]
                            nc.scalar.copy(dstv, src_)

                # ================= Phase C: attention =================
                with tc.tile_pool(name="scps", bufs=2, space="PSUM") as scp, \
                     tc.tile_pool(name="otps", bufs=2, space="PSUM") as otp, \
                     tc.tile_pool(name="wops", bufs=2, space="PSUM") as wop, \
                     tc.tile_pool(name="attsb", bufs=3) as asb, \
                     tc.tile_pool(name="outsb", bufs=3) as osb:
                    for qc in range(NCHUNK):
                        cq = qc // 2
                        qcol = (qc % 2) * 256
                        # block layout chosen so the exp'd region is one
                        # contiguous column range (valid parts: hi=[128:256),
                        # lo=[256:384), mid=[512:768) -> exp [128:768))
                        if qc == 0:
                            kts = [(1, "hi", 0), (0, "mid", 256)]
                            elo, ehi = 128, 512
                        else:
                            kts = [(2 * qc + 1, "hi", 0), (2 * qc - 1, "lo", 256),
                                   (2 * qc, "mid", 512)]
                            elo, ehi = 128, 768
                        stacked = [asb.tile([128, 256], F32R, tag=f"stk{p}", name=f"stk{p}")
                                   for p in range(2)]
                        pair_ots = [None, None]
                        for h in range(HPG):
                            m, hh = h // 2, h % 2
                            hp = slice(64 * hh, 64 * hh + 64)
                            sc = scp.tile([128, 768], F32, tag="sc", name="sc")
                            for kt, role, o in kts:
                                ck, kcol = kt // 4, (kt % 4) * 128
                                nc.tensor.matmul(sc[:, o:o + 256],
                                                 kr[m, ck][hp, kcol:kcol + 128],
                                                 qr[m, cq][hp, qcol:qcol + 256],
                                                 start=True, stop=True)
                            probs = asb.tile([128, 768], BF16, tag="probs", name="probs")
                            nc.scalar.activation(probs[:, elo:ehi], sc[:, elo:ehi],
                                                 EXP, bias=0.0, scale=SCALE)
                            # banded mask, multiplicative post-exp (keep iota >= 0);
                            # only regions the PV matmuls read get masked
                            for kt, role, o in kts:
                                if role == "lo":
                                    nc.gpsimd.affine_select(probs[:, o:o + 128], probs[:, o:o + 128],
                                                            pattern=[[-1, 128]], compare_op=GE,
                                                            fill=0.0, base=0, channel_multiplier=1)
                                elif role == "mid":
                                    nc.gpsimd.affine_select(probs[:, o:o + 128], probs[:, o:o + 128],
                                                            pattern=[[1, 128]], compare_op=GE,
                                                            fill=0.0, base=0, channel_multiplier=-1)
                                    nc.gpsimd.affine_select(probs[:, o + 128:o + 256], probs[:, o + 128:o + 256],
                                                            pattern=[[-1, 128]], compare_op=GE,
                                                            fill=0.0, base=0, channel_multiplier=1)
                                else:  # hi
                                    nc.gpsimd.affine_select(probs[:, o + 128:o + 256], probs[:, o + 128:o + 256],
                                                            pattern=[[1, 128]], compare_op=GE,
                                                            fill=0.0, base=0, channel_multiplier=-1)
                            # PV with ones-augment: rows 0:64 = oT_h, rows 64:128 = sums.
                            # mid tile first (start=True, full 256 cols); lo/hi add
                            # their valid 128-col halves.
                            if hh == 0:
                                pair_ots[m] = otp.tile([128, 512], F32, tag="ot", name="ot")
                            ot = pair_ots[m][:, hh * 256:(hh + 1) * 256]
                            pv_order = [e for e in kts if e[1] == "mid"] + \
                                       [e for e in kts if e[1] != "mid"]
                            for j, (kt, role, o) in enumerate(pv_order):
                                if role == "mid":
                                    rhs = probs[:, o:o + 256]
                                    dst = ot
                                elif role == "lo":
                                    rhs = probs[:, o:o + 128]
                                    dst = pair_ots[m][:, hh * 256:hh * 256 + 128]
                                else:
                                    rhs = probs[:, o + 128:o + 256]
                                    dst = pair_ots[m][:, hh * 256 + 128:(hh + 1) * 256]
                                nc.tensor.matmul(dst, v_t[kt][:, h * 128:(h + 1) * 128],
                                                 rhs, start=(j == 0), stop=(j == len(pv_order) - 1))
                            if hh == 1:
                                rbc = asb.tile([64, 512], F32, tag="rbc", name="rbc")
                                nc.vector.reciprocal(rbc[:], pair_ots[m][64:128, :])
                                nc.vector.tensor_tensor(stacked[m][0:64, :],
                                                        pair_ots[m][0:64, 0:256],
                                                        rbc[:, 0:256], MULT)
                                stg = asb.tile([64, 256], F32R, tag="stg", name="stg")
                                nc.vector.tensor_tensor(stg[:], pair_ots[m][0:64, 256:512],
                                                        rbc[:, 256:512], MULT)
                                nc.sync.dma_start(stacked[m][64:128, :], stg[:])
                        # output projection for this q-chunk
                        for tc2 in range(2):
                            trows = qc * 256 + tc2 * 128
                            ob = osb.tile([128, 1024], FP16, tag="ob", name="ob")
                            for nh in range(2):
                                wps = wop.tile([128, 512], F32, tag="wps", name="wps")
                                for p in range(2):
                                    nc.tensor.matmul(wps[:], stacked[p][:, tc2 * 128:tc2 * 128 + 128],
                                                     wo_t[p][:, nh * 512:(nh + 1) * 512],
                                                     start=(p == 0), stop=(p == 1))
                                nc.scalar.copy(ob[:, nh * 512:(nh + 1) * 512], wps[:])
                            nc.sync.dma_start(out[trows:trows + 128, :], ob[:])

    nc.compile()
    return nc


def _prep_inputs(x, rope_cos, rope_sin, wq, wk, wv, wo):
    perm = _chan_perm()
    pairs = np.array([_pair_of(j) for j in range(HD)])
    sgn = np.where((np.arange(HD) % 32) < 16, 1.0, -1.0).astype(np.float32)

    # (128, T) rope tiles in de-interleaved layout; identical for both 2-head tiles
    j64 = np.arange(128) % HD
    cos_t = np.ascontiguousarray(rope_cos.T[pairs[j64], :].astype(np.float32))
    sin_t = np.ascontiguousarray(
        (rope_sin.T[pairs[j64], :] * sgn[j64][:, None]).astype(np.float32))

    ins = []
    for b in range(B):
        xTb = np.ascontiguousarray(x[b].T)                          # (D, T)
        for g in range(G):
            rows = np.concatenate([g * C + h * HD + perm for h in range(HPG)])
            wqTg = np.ascontiguousarray(wq[rows, :].T)              # (D, C)
            wkTg = np.ascontiguousarray(wk[rows, :].T)
            wvTg = np.ascontiguousarray(wv[g * C:(g + 1) * C, :].T)  # (D, C)
            woTg = np.ascontiguousarray(wo[:, g * C:(g + 1) * C].T)  # (C, D)
            ins.append({
                "xT": xTb, "wqT": wqTg, "wkT": wkTg, "wvT": wvTg, "woT": woTg,
                "cosT": cos_t, "sinT": sin_t,
            })
    return ins


def kernel(x, rope_cos, rope_sin, wq, wk, wv, wo, _trace=False):
    from concourse.bass_utils import run_bass_kernel_spmd

    if "nc" not in _cache:
        _cache["nc"] = _build_program()
    nc = _cache["nc"]

    ins = _prep_inputs(np.asarray(x, np.float32), np.asarray(rope_cos, np.float32),
                       np.asarray(rope_sin, np.float32), np.asarray(wq, np.float32),
                       np.asarray(wk, np.float32), np.asarray(wv, np.float32),
                       np.asarray(wo, np.float32))
    kwargs = {}
    if _trace:
        kwargs = dict(trace=True)
    res = run_bass_kernel_spmd(nc, ins, core_ids=list(range(8)), **kwargs)
    _cache["last_result"] = res

    out = np.zeros((B, T, D), dtype=np.float32)
    for i in range(8):
        out[i // G] += res.results[i]["out"].astype(np.float32)
    return out



# revision 2
# speedup vs baseline: 1.2165x; 1.2165x over previous
"""Sliding-window causal attention with RoPE, distributed over 8 NeuronCores.

Sharding: 8 cores = (batch b in {0,1}) x (head-group g in {0..3}); each core
computes its batch's attention for 4 heads (256 channels) plus that group's
partial of the output projection; the host sums the 4 partials per batch.

Optimizations over the fp32r baseline (which was exactly Tensor-bound at
170,496 PE cycles ~ measured steady-state):
  - Q/K projections as fp8e4m3 DoubleRow matmuls (2x MAC rate; x/wq/wk
    pre-cast host-side, weight x64 scale folded into the exp scale).
  - scores lo/hi k-tiles compute only their 128 valid q-columns; qr/kr in
    bf16 so N=128 matmuls keep the 1 cycle/column rate.
  - projection chunks B(c) are interleaved between attention chunk-pairs
    (B0 B1 C0 C1 B2 C2 C3 B3 C4..C7), the output projection of chunk qc-1
    is emitted between scores(qc) and PV(qc) to cover the exp+mask latency,
    and within B the Q/K and V matmul groups alternate so V work hides the
    RoPE (DVE) drain of the Q/K PSUM ring.
  - banded masks: heads 0-1 via gpsimd affine_select, heads 2-3 via one DVE
    multiply with a precomputed [tri_le|tri_ge|tri_ge|tri_le] tile.
  - PSUM rings (bank-granular, 8 total): proj/wps shared ring 2 + V ring 2
    + scores ring 2 + pair_ots ring 2.
Per-core PE floor: 32768 (QK-DR) + 32768 (V) + 15872 (scores) + 15872 (PV)
+ 32768 (WO) = 130,048 cycles ~ 54.2us @2.4GHz.
"""

import numpy as np

B, T, D = 2, 2048, 1024
H, HD = 16, 64
G = 4            # head groups (cores per batch)
HPG = H // G     # heads per group = 4
C = HPG * HD     # channels per group = 256
SCALE = 0.125
WS = 64.0        # fp8 weight pre-scale (q,k each x64 -> scores x4096)
W = 128          # window per side
NQ = 256         # query chunk
NCHUNK = T // NQ
KT = T // 128    # k tiles

_cache = {}


def _chan_perm():
    # within-head permutation: pair i=(2i,2i+1) -> block layout where rows
    # [0:16)=re(0..15), [16:32)=im(0..15), [32:48)=re(16..31), [48:64)=im(16..31)
    perm = np.zeros(HD, dtype=np.int64)
    for j in range(HD):
        if j < 16:
            perm[j] = 2 * j
        elif j < 32:
            perm[j] = 2 * (j - 16) + 1
        elif j < 48:
            perm[j] = 2 * (j - 16)
        else:
            perm[j] = 2 * (j - 32) + 1
    return perm


def _pair_of(j):
    return (j % 16) + 16 * (j // 32)


def _build_program(repeat=1):
    import concourse.mybir as mybir
    import concourse.tile as tile
    from concourse import bacc

    F32 = mybir.dt.float32
    BF16 = mybir.dt.bfloat16
    FP16 = mybir.dt.float16
    F8 = mybir.dt.float8e4
    DR = mybir.MatmulPerfMode.DoubleRow
    MULT = mybir.AluOpType.mult
    ADD = mybir.AluOpType.add
    GE = mybir.AluOpType.is_ge
    EXP = mybir.ActivationFunctionType.Exp

    nc = bacc.Bacc("TRN2", target_bir_lowering=False, debug=False, num_devices=8)

    xT8 = nc.dram_tensor("xT8", (D, T), F8, kind="ExternalInput")
    xTb = nc.dram_tensor("xTb", (D, T), BF16, kind="ExternalInput")
    wqT = nc.dram_tensor("wqT", (D, C), F8, kind="ExternalInput")
    wkT = nc.dram_tensor("wkT", (D, C), F8, kind="ExternalInput")
    wvT = nc.dram_tensor("wvT", (D, C), BF16, kind="ExternalInput")
    woT = nc.dram_tensor("woT", (C, D), BF16, kind="ExternalInput")
    cosT = nc.dram_tensor("cosT", (128, T), BF16, kind="ExternalInput")
    sinT = nc.dram_tensor("sinT", (128, T), BF16, kind="ExternalInput")
    # banded-mask constants: [tri_le | tri_ge | tri_ge | tri_le] (512 cols)
    # and the qc=0 variant [tri_le | tri_ge | tri_le] (384 cols)
    mskT = nc.dram_tensor("mskT", (128, 512), BF16, kind="ExternalInput")
    msk0T = nc.dram_tensor("msk0T", (128, 384), BF16, kind="ExternalInput")
    out = nc.dram_tensor("out", (T, D), FP16, kind="ExternalOutput")

    shuf16 = [(i + 16) % 32 for i in range(32)]
    ESCALE = SCALE / (WS * WS)

    with tile.TileContext(nc) as tc:
        with tc.tile_pool(name="persist", bufs=1) as pp:
            # ---- loads, ordered so the first projection chunk's operands
            # land first
            wq_a = pp.tile([128, 8, C], F8, tag="wq_a", name="wq_a")
            x8_t = {}
            x8_t[0, 0] = pp.tile([128, 2, 512], F8, tag="x8_00", name="x8_00")
            nc.sync.dma_start(x8_t[0, 0][:],
                              xT8[0:256, 0:512].rearrange("(g p) t -> p g t", p=128))
            nc.sync.dma_start(wq_a[:], wqT.rearrange("(g p) c -> p g c", p=128))
            for kp in range(1, 4):
                x8_t[kp, 0] = pp.tile([128, 2, 512], F8, tag=f"x8_{kp}0",
                                      name=f"x8_{kp}0")
                nc.sync.dma_start(
                    x8_t[kp, 0][:],
                    xT8[kp * 256:(kp + 1) * 256, 0:512].rearrange(
                        "(g p) t -> p g t", p=128))
            wk_a = pp.tile([128, 8, C], F8, tag="wk_a", name="wk_a")
            nc.sync.dma_start(wk_a[:], wkT.rearrange("(g p) c -> p g c", p=128))
            cos_c, sin_c = [], []
            for c in range(4):
                cc = pp.tile([128, 512], BF16, tag=f"cos{c}", name=f"cos{c}")
                sc_ = pp.tile([128, 512], BF16, tag=f"sin{c}", name=f"sin{c}")
                if c == 0:
                    nc.sync.dma_start(cc[:], cosT[:, 0:512])
                    nc.sync.dma_start(sc_[:], sinT[:, 0:512])
                cos_c.append(cc)
                sin_c.append(sc_)
            xb_t = {}
            for k in range(8):
                xb_t[k, 0] = pp.tile([128, 512], BF16, tag=f"xb{k}_0",
                                     name=f"xb{k}_0")
                nc.sync.dma_start(xb_t[k, 0][:], xTb[k * 128:(k + 1) * 128, 0:512])
            wv_a = pp.tile([128, 8, C], BF16, tag="wv_a", name="wv_a")
            nc.sync.dma_start(wv_a[:], wvT.rearrange("(g p) c -> p g c", p=128))
            wv_t = [wv_a[:, k, :] for k in range(8)]
            msk = pp.tile([128, 512], BF16, tag="msk", name="msk")
            nc.sync.dma_start(msk[:], mskT[:, :])
            msk0 = pp.tile([128, 384], BF16, tag="msk0", name="msk0")
            nc.sync.dma_start(msk0[:], msk0T[:, :])
            for c in range(1, 4):
                nc.sync.dma_start(cos_c[c][:], cosT[:, c * 512:(c + 1) * 512])
                nc.sync.dma_start(sin_c[c][:], sinT[:, c * 512:(c + 1) * 512])
                for kp in range(4):
                    x8_t[kp, c] = pp.tile([128, 2, 512], F8, tag=f"x8_{kp}{c}",
                                          name=f"x8_{kp}{c}")
                    nc.sync.dma_start(
                        x8_t[kp, c][:],
                        xT8[kp * 256:(kp + 1) * 256,
                            c * 512:(c + 1) * 512].rearrange(
                                "(g p) t -> p g t", p=128))
                for k in range(8):
                    xb_t[k, c] = pp.tile([128, 512], BF16, tag=f"xb{k}_{c}",
                                         name=f"xb{k}_{c}")
                    nc.sync.dma_start(xb_t[k, c][:],
                                      xTb[k * 128:(k + 1) * 128,
                                          c * 512:(c + 1) * 512])
            wo_a = pp.tile([128, 2, D], BF16, tag="wo_a", name="wo_a")
            nc.sync.dma_start(wo_a[:], woT.rearrange("(g p) c -> p g c", p=128))
            wo_t = [wo_a[:, k, :] for k in range(2)]

            # ---- persistent activation storage
            qr = {}
            kr = {}
            for m in range(2):
                for c in range(4):
                    qr[m, c] = pp.tile([128, 512], BF16, tag=f"qr{m}_{c}",
                                       name=f"qr{m}_{c}")
                    kr[m, c] = pp.tile([128, 512], BF16, tag=f"kr{m}_{c}",
                                       name=f"kr{m}_{c}")
            v_t = [pp.tile([128, 512], BF16, tag=f"v{t}", name=f"v{t}")
                   for t in range(KT)]
            for t in range(KT):
                nc.gpsimd.memset(v_t[t][:], 1.0)

            # PSUM rings (bank-granular): proj+wps 2 + V 2 + scores 2 +
            # pair_ots 2 = 8 banks.
            with tc.tile_pool(name="projps", bufs=2, space="PSUM") as pjp, \
                 tc.tile_pool(name="projvps", bufs=1, space="PSUM") as pjv, \
                 tc.tile_pool(name="scps", bufs=3, space="PSUM") as scp, \
                 tc.tile_pool(name="otps", bufs=2, space="PSUM") as otp, \
                 tc.tile_pool(name="ropetmp", bufs=6) as rtp, \
                 tc.tile_pool(name="attsb", bufs=4) as asb, \
                 tc.tile_pool(name="outsb", bufs=3) as osb:

                def emit_B(c):
                    """Projection chunk c: Q/K (fp8 DR + rope) and V, with
                    the V matmul groups interleaved between Q/K groups so PE
                    keeps running while DVE drains the Q/K PSUM ring."""
                    groups = []
                    for (wa, dst) in ((wq_a, qr), (wk_a, kr)):
                        for m in range(2):
                            groups.append(("qk", wa, dst, m))
                    for tt in range(c * 4, c * 4 + 4):
                        groups.append(("v", tt))
                    order = [0, 4, 1, 5, 2, 6, 3, 7]
                    for gi in order:
                        g = groups[gi]
                        if g[0] == "qk":
                            _, wa, dst, m = g
                            ps = pjp.tile([128, 512], F32, tag="proj",
                                          name="proj")
                            for kp in range(4):
                                nc.tensor.matmul(
                                    ps[:],
                                    wa[:, 2 * kp:2 * kp + 2,
                                       m * 128:(m + 1) * 128],
                                    x8_t[kp, c][:],
                                    start=(kp == 0), stop=(kp == 3),
                                    perf_mode=DR)
                            psb = rtp.tile([128, 512], BF16, tag="psb",
                                           name="psb")
                            nc.scalar.copy(psb[:], ps[:])
                            z = rtp.tile([128, 512], BF16, tag="ropez",
                                         name="ropez")
                            nc.vector.tensor_tensor(z[:], psb[:], sin_c[c][:],
                                                    MULT)
                            zs = rtp.tile([128, 512], BF16, tag="ropezs",
                                          name="ropezs")
                            nc.vector.stream_shuffle(zs[:], z[:], shuf16)
                            t1 = rtp.tile([128, 512], BF16, tag="ropet1",
                                          name="ropet1")
                            nc.vector.tensor_tensor(t1[:], psb[:], cos_c[c][:],
                                                    MULT)
                            nc.vector.tensor_tensor(dst[m, c][:], t1[:],
                                                    zs[:], ADD)
                        else:
                            tt = g[1]
                            if tt % 2 == 0:
                                psv2 = pjv.tile([128, 2, C], F32, tag="projv",
                                                name="projv")
                                emitq = getattr(emit_B, "_psv", {})
                                emitq[c] = psv2
                                emit_B._psv = emitq
                            psv = emit_B._psv[c][:, tt % 2, :]
                            for k in range(8):
                                nc.tensor.matmul(
                                    psv,
                                    xb_t[k, c][:, (tt % 4) * 128:(tt % 4) * 128 + 128],
                                    wv_t[k][:],
                                    start=(k == 0), stop=(k == 7))
                            src_ = psv.rearrange("p (h d) -> p h d", h=HPG)
                            dstv = v_t[tt][:].rearrange(
                                "p (h d) -> p h d", h=HPG)[:, :, 0:64]
                            nc.scalar.copy(dstv, src_)

                def emit_WO(stacked, qc):
                    for tc2 in range(2):
                        trows = qc * 256 + tc2 * 128
                        ob = osb.tile([128, 1024], FP16, tag="ob", name="ob")
                        for nh in range(2):
                            wps = pjp.tile([128, 512], F32, tag="proj",
                                           name="wps")
                            for p in range(2):
                                nc.tensor.matmul(
                                    wps[:],
                                    stacked[p][:, tc2 * 128:tc2 * 128 + 128],
                                    wo_t[p][:, nh * 512:(nh + 1) * 512],
                                    start=(p == 0), stop=(p == 1))
                            nc.scalar.copy(ob[:, nh * 512:(nh + 1) * 512],
                                           wps[:])
                        nc.sync.dma_start(out[trows:trows + 128, :], ob[:])

                pending_wo = None
                for _rep in range(repeat):
                    for qc in range(NCHUNK):
                        if qc == 0:
                            emit_B(0)
                            emit_B(1)
                        elif qc in (2, 4):
                            emit_B(qc // 2 + 1)
                        cq = qc // 2
                        qcol = (qc % 2) * 256
                        # probs layout: [mid 0:256][lo 256:384][hi 384:512]
                        # (qc=0: no lo, hi at 256:384); lo/hi carry only
                        # their 128 valid q-columns.
                        if qc == 0:
                            kts = [(0, "mid", 0), (1, "hi", 256)]
                            ehi = 384
                        else:
                            kts = [(2 * qc, "mid", 0), (2 * qc - 1, "lo", 256),
                                   (2 * qc + 1, "hi", 384)]
                            ehi = 512
                        stacked = [asb.tile([128, 256], BF16, tag=f"stk{p}",
                                            name=f"stk{p}") for p in range(2)]
                        probs_t = {}
                        # scores + exp + mask for all 4 heads
                        for h in range(HPG):
                            m, hh = h // 2, h % 2
                            hp = slice(64 * hh, 64 * hh + 64)
                            sc = scp.tile([128, 512], F32, tag="sc", name="sc")
                            for kt, role, o in kts:
                                ck, kcol = kt // 4, (kt % 4) * 128
                                if role == "mid":
                                    rhs = qr[m, cq][hp, qcol:qcol + 256]
                                    dst = sc[:, o:o + 256]
                                elif role == "lo":
                                    rhs = qr[m, cq][hp, qcol:qcol + 128]
                                    dst = sc[:, o:o + 128]
                                else:
                                    rhs = qr[m, cq][hp, qcol + 128:qcol + 256]
                                    dst = sc[:, o:o + 128]
                                nc.tensor.matmul(dst,
                                                 kr[m, ck][hp, kcol:kcol + 128],
                                                 rhs, start=True, stop=True)
                            probs = asb.tile([128, 512], BF16, tag="probs",
                                             name="probs")
                            probs_t[h] = probs
                            nc.scalar.activation(probs[:, 0:ehi], sc[:, 0:ehi],
                                                 EXP, bias=0.0, scale=ESCALE)
                            # banded mask, multiplicative post-exp. Local col
                            # j, k-row r: mid[0:128]/hi keep r<=j,
                            # mid[128:256]/lo keep r>=j. Heads 0-1 on gpsimd,
                            # heads 2-3 as one DVE multiply.
                            if h < 2:
                                for kt, role, o in kts:
                                    if role == "mid":
                                        nc.gpsimd.affine_select(
                                            probs[:, o:o + 128],
                                            probs[:, o:o + 128],
                                            pattern=[[1, 128]], compare_op=GE,
                                            fill=0.0, base=0,
                                            channel_multiplier=-1)
                                        nc.gpsimd.affine_select(
                                            probs[:, o + 128:o + 256],
                                            probs[:, o + 128:o + 256],
                                            pattern=[[-1, 128]], compare_op=GE,
                                            fill=0.0, base=0,
                                            channel_multiplier=1)
                                    elif role == "lo":
                                        nc.gpsimd.affine_select(
                                            probs[:, o:o + 128],
                                            probs[:, o:o + 128],
                                            pattern=[[-1, 128]], compare_op=GE,
                                            fill=0.0, base=0,
                                            channel_multiplier=1)
                                    else:  # hi
                                        nc.gpsimd.affine_select(
                                            probs[:, o:o + 128],
                                            probs[:, o:o + 128],
                                            pattern=[[1, 128]], compare_op=GE,
                                            fill=0.0, base=0,
                                            channel_multiplier=-1)
                            else:
                                mt = msk[:, 0:512] if qc > 0 else msk0[:, 0:384]
                                nc.vector.tensor_tensor(probs[:, 0:ehi],
                                                        probs[:, 0:ehi],
                                                        mt, MULT)
                        # previous chunk's output projection: PE work that
                        # covers this chunk's exp+mask latency
                        if pending_wo is not None:
                            emit_WO(*pending_wo)
                        # PV with ones-augment: rows 0:64 = oT_h, 64:128 sums
                        pair_ots = [None, None]
                        for h in range(HPG):
                            m, hh = h // 2, h % 2
                            probs = probs_t[h]
                            if hh == 0:
                                pair_ots[m] = otp.tile([128, 512], F32,
                                                       tag="ot", name="ot")
                            ot = pair_ots[m][:, hh * 256:(hh + 1) * 256]
                            last = len(kts) - 1
                            for j, (kt, role, o) in enumerate(kts):
                                if role == "mid":
                                    rhs = probs[:, o:o + 256]
                                    dst = ot
                                elif role == "lo":
                                    rhs = probs[:, o:o + 128]
                                    dst = pair_ots[m][:, hh * 256:hh * 256 + 128]
                                else:
                                    rhs = probs[:, o:o + 128]
                                    dst = pair_ots[m][:, hh * 256 + 128:
                                                      (hh + 1) * 256]
                                nc.tensor.matmul(dst,
                                                 v_t[kt][:, h * 128:(h + 1) * 128],
                                                 rhs, start=(j == 0),
                                                 stop=(j == last))
                            if hh == 1:
                                rbc = asb.tile([64, 512], F32, tag="rbc",
                                               name="rbc")
                                nc.vector.reciprocal(rbc[:],
                                                     pair_ots[m][64:128, :])
                                nc.vector.tensor_tensor(stacked[m][0:64, :],
                                                        pair_ots[m][0:64, 0:256],
                                                        rbc[:, 0:256], MULT)
                                stg = asb.tile([64, 256], BF16, tag="stg",
                                               name="stg")
                                nc.vector.tensor_tensor(stg[:],
                                                        pair_ots[m][0:64, 256:512],
                                                        rbc[:, 256:512], MULT)
                                nc.sync.dma_start(stacked[m][64:128, :], stg[:])
                        pending_wo = (stacked, qc)
                if pending_wo is not None:
                    emit_WO(*pending_wo)

    nc.compile()
    return nc


def _prep_inputs(x, rope_cos, rope_sin, wq, wk, wv, wo):
    import ml_dtypes
    F8 = ml_dtypes.float8_e4m3
    BF = ml_dtypes.bfloat16

    perm = _chan_perm()
    pairs = np.array([_pair_of(j) for j in range(HD)])
    sgn = np.where((np.arange(HD) % 32) < 16, 1.0, -1.0).astype(np.float32)

    # (128, T) rope tiles in de-interleaved layout; identical for both 2-head
    # tiles
    j64 = np.arange(128) % HD
    cos_t = np.ascontiguousarray(rope_cos.T[pairs[j64], :].astype(BF))
    sin_t = np.ascontiguousarray(
        (rope_sin.T[pairs[j64], :] * sgn[j64][:, None]).astype(BF))

    r = np.arange(128)[:, None]
    j = np.arange(128)[None, :]
    tri_le = (r <= j).astype(np.float32)
    tri_ge = (r >= j).astype(np.float32)
    msk = np.ascontiguousarray(
        np.concatenate([tri_le, tri_ge, tri_ge, tri_le], axis=1).astype(BF))
    msk0 = np.ascontiguousarray(
        np.concatenate([tri_le, tri_ge, tri_le], axis=1).astype(BF))

    ins = []
    for b in range(B):
        xTb = np.ascontiguousarray(x[b].T)                       # (D, T)
        xT8 = xTb.astype(F8)
        xTbf = xTb.astype(BF)
        for g in range(G):
            rows = np.concatenate([g * C + h * HD + perm for h in range(HPG)])
            wq8 = np.ascontiguousarray((wq[rows, :] * WS).T.astype(F8))
            wk8 = np.ascontiguousarray((wk[rows, :] * WS).T.astype(F8))
            wvTg = np.ascontiguousarray(
                wv[g * C:(g + 1) * C, :].T.astype(BF))               # (D, C)
            woTg = np.ascontiguousarray(
                wo[:, g * C:(g + 1) * C].T.astype(BF))               # (C, D)
            ins.append({
                "xT8": xT8, "xTb": xTbf, "wqT": wq8, "wkT": wk8,
                "wvT": wvTg, "woT": woTg, "cosT": cos_t, "sinT": sin_t,
                "mskT": msk, "msk0T": msk0,
            })
    return ins


def kernel(x, rope_cos, rope_sin, wq, wk, wv, wo, _trace=False):
    from concourse.bass_utils import run_bass_kernel_spmd

    if "nc" not in _cache:
        _cache["nc"] = _build_program()
    nc = _cache["nc"]

    ins = _prep_inputs(np.asarray(x, np.float32), np.asarray(rope_cos, np.float32),
                       np.asarray(rope_sin, np.float32), np.asarray(wq, np.float32),
                       np.asarray(wk, np.float32), np.asarray(wv, np.float32),
                       np.asarray(wo, np.float32))
    kwargs = {}
    if _trace:
        kwargs = dict(trace=True)
    res = run_bass_kernel_spmd(nc, ins, core_ids=list(range(8)), **kwargs)
    _cache["last_result"] = res

    out = np.zeros((B, T, D), dtype=np.float32)
    for i in range(8):
        out[i // G] += res.results[i]["out"].astype(np.float32)
    return out
